# revision 1
# baseline (speedup 1.0000x reference)
"""Trainium2 Bass kernel for nn_CaptionModel (GRU + Bahdanau attention caption decoder).

Sharding: pure data-parallel over batch. B=64 -> 8 cores x 8 rows each; no
collectives (50 sequential steps cannot afford the ~5us/call collective floor).

Per-core plan (feature-major: features on partitions, local batch b=8 on free):
  setup:  enc = W_feat.T @ spatialT (+b_feat)        [512, 392]
          att1 = W_ea.T @ enc (+b_ea)                [256, 392] bf16
          enc_bd: block-diag [l, d] pair tiles for the context matmuls
          gi_emb = W_ih[:, :EMB].T @ embT (+biases)  [1536, 400] f32
  50 steps (weight-stationary matmuls, bf16 weights):
          gh   = W_hh.T @ h            (48 mm)
          att2 = W_da.T @ h (+b_da)    (8 mm)
          tanh(att1 + att2) -> scores = tanhT @ W_fa  (per-b mm into [l-part, b-col] psum)
          exp -> denom (ones mm) -> 1/denom -> broadcast (ones outer-product mm)
          context (block-diag mm) -> x_ctx = ctx * rinv
          gi_ctx = W_ihc.T @ x_ctx     (48 mm, accumulated with gh in psum for r,z)
          gates: sigmoid via 0.5+0.5*tanh(x/2) (single ACT table set: exp+tanh)
          h stored fp32; bf16 copy appended to H_hist
  tail:   logits = H_hist.T @ W_fc + b_fc, batch-major out, streamed to DRAM.

kernel() accepts FULL inputs, does host-side layout prep/sharding (incl. the
embedding-table gather), runs the same NEFF SPMD on cores 0-7, unshards.
"""

import contextlib

import ml_dtypes
import numpy as np

import concourse.bass as bass
import concourse.mybir as mybir
from concourse import bacc
from concourse.alu_op_type import AluOpType as Op
from concourse.masks import make_identity
from concourse.tile import TileContext

AF = mybir.ActivationFunctionType
F32 = mybir.dt.float32
BF16 = mybir.dt.bfloat16
F32R = mybir.dt.float32r

B, L, ENC, DEC, EMB, ATT, V, T = 64, 49, 2048, 512, 512, 256, 10000, 50
NCORES = 8
BL = B // NCORES          # 8 local batch rows
NL = BL * L               # 392
KE = ENC // 128           # 16 K-chunks for enc matmul
KD = DEC // 128           # 4 K-chunks over DEC
MG = (3 * DEC) // 128     # 12 M-chunks over gates
MA = ATT // 128           # 2 M-chunks over ATT
NPAIR = BL // 2           # 4 block-diag pairs
HCOL = 8 * (T + 1)        # 408 cols per chunk in H history
NV = 512                  # fc vocab tile width
NCK = (V + NV - 1) // NV  # 20 fc vocab tiles (last = 272 wide)


def build_program(n_steps=T, do_rec=True, do_fc=True):
    nc = bacc.Bacc()
    NT = BL * n_steps      # t*8+b columns
    hcol = 8 * (n_steps + 1)

    # ---------------- DRAM I/O (per-core, host-prepped layouts) ----------------
    d_spatialT = nc.dram_tensor("spatialT", [128, KE * NL], BF16, kind="ExternalInput")
    d_embT = nc.dram_tensor("embT", [128, KD * NT], BF16, kind="ExternalInput")
    d_wfeat = nc.dram_tensor("wfeat", [128, KE * DEC], BF16, kind="ExternalInput")
    d_wea = nc.dram_tensor("wea", [128, KD * ATT], BF16, kind="ExternalInput")
    d_wihe = nc.dram_tensor("wihe", [128, KD * 3 * DEC], BF16, kind="ExternalInput")
    d_wihc = nc.dram_tensor("wihc", [128, KD * MG * 128], BF16, kind="ExternalInput")
    d_whh = nc.dram_tensor("whh", [128, KD * MG * 128], BF16, kind="ExternalInput")
    d_wda = nc.dram_tensor("wda", [128, KD * MA * 128], BF16, kind="ExternalInput")
    d_wfa = nc.dram_tensor("wfa", [128, MA], BF16, kind="ExternalInput")
    d_wfc = nc.dram_tensor("wfc", [128, KD * V], BF16, kind="ExternalInput")
    d_bfeat = nc.dram_tensor("bfeat", [128, KD], F32, kind="ExternalInput")
    d_bea = nc.dram_tensor("bea", [128, MA], F32, kind="ExternalInput")
    d_biasgi = nc.dram_tensor("biasgi", [128, MG], F32, kind="ExternalInput")
    d_bhhnbc = nc.dram_tensor("bhhnbc", [128, 4 * BL], BF16, kind="ExternalInput")
    d_bfc = nc.dram_tensor("bfc", [1, V], BF16, kind="ExternalInput")
    d_logits = nc.dram_tensor("logits", [NT, V], BF16, kind="ExternalOutput")

    with TileContext(nc) as tc, contextlib.ExitStack() as ctx:
        const = ctx.enter_context(tc.tile_pool(name="const", bufs=1))
        state = ctx.enter_context(tc.tile_pool(name="state", bufs=1))

        # persistent weights / constants in SBUF
        wihc = const.tile([128, KD * MG * 128], BF16, tag="wihc")
        whh = const.tile([128, KD * MG * 128], BF16, tag="whh")
        wda = const.tile([128, KD * MA * 128], BF16, tag="wda")
        wfa = const.tile([128, MA], BF16, tag="wfa")
        bhhnbc = const.tile([128, 4 * BL], BF16, tag="bhhnbc")
        biasgi = const.tile([128, MG], F32, tag="biasgi")
        ident_f = const.tile([128, 128], BF16, tag="ident_f")
        make_identity(nc, ident_f[:])
        ones_mat_bf = const.tile([128, 128], BF16, tag="ones_mat")
        bfcb = const.tile([128, V], BF16, tag="bfcb")
        wfc_sb = const.tile([128, KD * V], BF16, tag="wfc_sb")
        for dst, src in [(wihc, d_wihc), (whh, d_whh), (wda, d_wda), (wfa, d_wfa),
                         (bhhnbc, d_bhhnbc), (biasgi, d_biasgi)]:
            nc.sync.dma_start(dst[:], src[:])
        nc.gpsimd.memset(ones_mat_bf[:], 1.0)
        nc.sync.dma_start(bfcb[:], d_bfc[:].partition_broadcast(128))

        # persistent activations / state
        att1 = state.tile([128, MA * NL], BF16, tag="att1")
        enc_bd = [state.tile([128, KD * 128], BF16, tag=f"encbd{j}", name=f"encbd{j}") for j in range(NPAIR)]
        gi_emb = state.tile([128, MG * NT], BF16, tag="gi_emb")
        hist = state.tile([128, KD * hcol], BF16, tag="hist")
        expe = state.tile([128, BL], BF16, tag="expe")
        tanh_sb = state.tile([128, MA * BL * 64], BF16, tag="tanh_sb")
        nc.gpsimd.memset(tanh_sb[:], 0.0)
        nc.gpsimd.memset(hist[:], 0.0)
        nc.gpsimd.memset(expe[:], 0.0)
        for j in range(NPAIR):
            nc.gpsimd.memset(enc_bd[j][:], 0.0)

        # ------------------------------ setup phase ------------------------------
        with tc.tile_pool(name="ssb", bufs=1) as ssb, \
             tc.tile_pool(name="sps", bufs=2, space="PSUM") as sps:
            spatialT = ssb.tile([128, KE * NL], BF16, tag="spatialT")
            embT = ssb.tile([128, KD * NT], BF16, tag="embT")
            wfeat = ssb.tile([128, KE * DEC], BF16, tag="wfeat")
            wea = ssb.tile([128, KD * ATT], BF16, tag="wea")
            wihe = ssb.tile([128, KD * 3 * DEC], BF16, tag="wihe")
            bfeat = ssb.tile([128, KD], F32, tag="bfeat")
            bea = ssb.tile([128, MA], F32, tag="bea")
            for dst, src in [(spatialT, d_spatialT), (embT, d_embT), (wfeat, d_wfeat),
                             (wea, d_wea), (wihe, d_wihe), (bfeat, d_bfeat), (bea, d_bea)]:
                nc.sync.dma_start(dst[:], src[:])

            enc_fm = ssb.tile([128, KD * NL], BF16, tag="enc_fm")
            # enc = W_feat.T @ spatialT  (+ b_feat), feature-major [dec-chunk, b*49+l]
            for mc in range(KD):
                p = sps.tile([128, NL], F32, tag="p_enc")
                for kc in range(KE):
                    nc.tensor.matmul(
                        p[:],
                        wfeat[:, kc * DEC + mc * 128: kc * DEC + mc * 128 + 128],
                        spatialT[:, kc * NL: (kc + 1) * NL],
                        start=(kc == 0), stop=(kc == KE - 1))
                nc.vector.tensor_scalar(
                    enc_fm[:, mc * NL: (mc + 1) * NL], p[:],
                    bfeat[:, mc: mc + 1], None, Op.add)

            # att1 = W_ea.T @ enc (+ b_ea)  -> bf16 [att-chunk, b*49+l]
            for mc in range(MA):
                p = sps.tile([128, NL], F32, tag="p_att1")
                for kc in range(KD):
                    nc.tensor.matmul(
                        p[:],
                        wea[:, kc * ATT + mc * 128: kc * ATT + mc * 128 + 128],
                        enc_fm[:, kc * NL: (kc + 1) * NL],
                        start=(kc == 0), stop=(kc == KD - 1))
                nc.vector.tensor_scalar(
                    att1[:, mc * NL: (mc + 1) * NL], p[:],
                    bea[:, mc: mc + 1], None, Op.add)

            # enc_bd[j]: rows 0:49 = enc[b=2j] (l, d); rows 64:113 = enc[b=2j+1]
            # (psum/ACT partition bases must be 0/32/64 -> 64-offset pairing).
            for c in range(KD):
                for b in range(BL):
                    base = 64 * (b % 2)
                    pt = sps.tile([128, 128], F32, tag="p_tr")
                    nc.tensor.matmul(
                        pt[base: base + L, :],
                        enc_fm[:, c * NL + b * L: c * NL + b * L + L],
                        ident_f[:], start=True, stop=True)
                    nc.vector.tensor_copy(
                        enc_bd[b // 2][base: base + L, c * 128: (c + 1) * 128],
                        pt[base: base + L, :])

            # gi_emb = W_ihe.T @ embT (+ b_ih + [b_hh folded for r,z])
            for mc in range(MG):
                p = sps.tile([128, NT], F32, tag="p_gie")
                for kc in range(KD):
                    nc.tensor.matmul(
                        p[:],
                        wihe[:, kc * 3 * DEC + mc * 128: kc * 3 * DEC + mc * 128 + 128],
                        embT[:, kc * NT: (kc + 1) * NT],
                        start=(kc == 0), stop=(kc == KD - 1))
                nc.vector.tensor_scalar(
                    gi_emb[:, mc * NT: (mc + 1) * NT], p[:],
                    biasgi[:, mc: mc + 1], None, Op.add)

        # ------------------------------ recurrence ------------------------------
        nc.sync.dma_start(wfc_sb[:], d_wfc[:])
        with tc.tile_pool(name="rsb", bufs=3) as rsb, \
             tc.tile_pool(name="rps", bufs=1, space="PSUM") as rps:
            for t in range(n_steps if do_rec else 0):
                hprev = [hist[:, kc * hcol + 8 * t: kc * hcol + 8 * t + 8] for kc in range(KD)]

                # gh (r,z and n) and att2, weight-stationary bf16. The gi_emb
                # slice and b_hh_n enter psum via identity matmuls (frees the
                # DVE pre-add chain; ACT reads gates straight from psum).
                gie = gi_emb[:].rearrange("p (mc tb) -> p mc tb", mc=MG)
                p_ghrz = rps.tile([128, 64], F32, tag="p_ghrz", bufs=2)
                p_ghn = rps.tile([128, 32], F32, tag="p_ghn")
                p_att2 = rps.tile([128, MA * BL], F32, tag="p_att2")
                nc.tensor.matmul(
                    p_ghrz[:], ident_f[:],
                    gie[:, 0:8, 8 * t: 8 * t + 8],
                    start=True, stop=False)
                nc.tensor.matmul(
                    p_ghn[:], ident_f[:],
                    bhhnbc[:],
                    start=True, stop=False)
                for mc in range(MA):
                    for kc in range(KD):
                        nc.tensor.matmul(
                            p_att2[:, mc * 8: mc * 8 + 8],
                            wda[:, (kc * MA + mc) * 128: (kc * MA + mc) * 128 + 128],
                            hprev[kc], start=(kc == 0), stop=(kc == KD - 1))
                for mc in range(8):
                    for kc in range(KD):
                        nc.tensor.matmul(
                            p_ghrz[:, mc * 8: mc * 8 + 8],
                            whh[:, (kc * MG + mc) * 128: (kc * MG + mc) * 128 + 128],
                            hprev[kc], start=False, stop=False)
                for mc in range(8, MG):
                    for kc in range(KD):
                        nc.tensor.matmul(
                            p_ghn[:, (mc - 8) * 8: (mc - 8) * 8 + 8],
                            whh[:, (kc * MG + mc) * 128: (kc * MG + mc) * 128 + 128],
                            hprev[kc], start=False,
                            stop=(kc == KD - 1 and mc == MG - 1))

                # tanh(att1 + att2 broadcast over l); b_da+b_ea pre-folded into
                # att1. Split by att-chunk so the second TT overlaps the first
                # tanh (shorter critical path into the score matmuls).
                targ = rsb.tile([128, MA * NL], BF16, tag="targ")
                for c in range(MA):
                    nc.vector.tensor_tensor(
                        targ[:, c * NL: (c + 1) * NL].rearrange(
                            "p (b l) -> p b l", b=BL, l=L),
                        att1[:, c * NL: (c + 1) * NL].rearrange(
                            "p (b l) -> p b l", b=BL, l=L),
                        p_att2[:, c * BL: (c + 1) * BL].unsqueeze(2)
                        .broadcast_to([128, BL, L]),
                        Op.add)
                    nc.scalar.activation(
                        tanh_sb[:, c * 512: (c + 1) * 512].rearrange(
                            "p (b l) -> p b l", b=BL, l=64)[:, :, 0:L],
                        targ[:, c * NL: (c + 1) * NL].rearrange(
                            "p (b l) -> p b l", b=BL, l=L),
                        AF.Tanh)

                # scores -> psum [128 rows, pair cols]: one MM per (pair, kc);
                # lhsT is the 64-stride padded pair block (odd b -> rows 64+)
                p_sc = rps.tile([128, NPAIR], F32, tag="p_sc")
                for j in range(NPAIR):
                    for kc in range(MA):
                        nc.tensor.matmul(
                            p_sc[:, j: j + 1],
                            tanh_sb[:, kc * 512 + j * 128: kc * 512 + j * 128 + 128],
                            wfa[:, kc: kc + 1],
                            start=(kc == 0), stop=(kc == MA - 1))

                # exp (no max-subtraction: scores are tiny); block-diag layout kept zero
                nc.scalar.activation(expe[0:L, 0:BL:2], p_sc[0:L, 0:NPAIR], AF.Exp)
                nc.scalar.activation(expe[64:64 + L, 1:BL:2], p_sc[64:64 + L, 0:NPAIR], AF.Exp)

                # denominator broadcast to all partitions in one matmul
                # (lhsT = all-ones [128,128]), then reciprocal psum->sbuf
                p_small = rps.tile([128, BL], F32, tag="p_small")
                nc.tensor.matmul(p_small[:], ones_mat_bf[:], expe[:], start=True, stop=True)
                rb_sb = rsb.tile([128, BL], F32, tag="rb_sb")
                nc.vector.reciprocal(rb_sb[:], p_small[:])

                # context (block-diag pairs) and normalization
                p_ctx = rps.tile([128, KD * BL], F32, tag="p_ctx")
                for j in range(NPAIR):
                    for c in range(KD):
                        nc.tensor.matmul(
                            p_ctx[:, c * 8 + 2 * j: c * 8 + 2 * j + 2],
                            enc_bd[j][:, c * 128: (c + 1) * 128],
                            expe[:, 2 * j: 2 * j + 2],
                            start=True, stop=True)
                x_ctx = rsb.tile([128, KD * BL], BF16, tag="x_ctx")
                nc.vector.tensor_tensor(
                    x_ctx[:].rearrange("p (c b) -> p c b", c=KD),
                    p_ctx[:].rearrange("p (c b) -> p c b", c=KD),
                    rb_sb[:].unsqueeze(1).broadcast_to([128, KD, BL]),
                    Op.mult)

                # gi_ctx: r,z accumulate onto p_ghrz; n into p_gin (pre-loaded
                # with the gi_emb n-slice via identity matmul)
                p_gin = rps.tile([128, 32], F32, tag="p_gin")
                nc.tensor.matmul(
                    p_gin[:], ident_f[:],
                    gie[:, 8:MG, 8 * t: 8 * t + 8],
                    start=True, stop=False)
                for mc in range(8):
                    for kc in range(KD):
                        nc.tensor.matmul(
                            p_ghrz[:, mc * 8: mc * 8 + 8],
                            wihc[:, (kc * MG + mc) * 128: (kc * MG + mc) * 128 + 128],
                            x_ctx[:, kc * 8: kc * 8 + 8], start=False,
                            stop=(kc == KD - 1 and mc == 7))
                for mc in range(8, MG):
                    for kc in range(KD):
                        nc.tensor.matmul(
                            p_gin[:, (mc - 8) * 8: (mc - 8) * 8 + 8],
                            wihc[:, (kc * MG + mc) * 128: (kc * MG + mc) * 128 + 128],
                            x_ctx[:, kc * 8: kc * 8 + 8], start=False,
                            stop=(kc == KD - 1 and mc == MG - 1))

                # gates: t_rz = tanh(0.5 * rz_full) straight from psum
                t_rz = rsb.tile([128, 64], F32, tag="t_rz")
                nc.scalar.activation(t_rz[:], p_ghrz[:], AF.Tanh, scale=0.5)
                # r' and z' sigmoids in one affine op: 0.5*t + 0.5. The
                # n-gate chain (vv -> n_arg -> tanh) is the critical path, so
                # it issues on DVE before the off-chain zm/w1 ops, which then
                # execute under the ACT tanh.
                trz1 = rsb.tile([128, 64], F32, tag="trz1")
                nc.vector.tensor_scalar(trz1[:], t_rz[:], 0.5, 0.5, Op.mult, Op.add)
                vv = rsb.tile([128, 32], F32, tag="vv")
                nc.vector.tensor_tensor(vv[:], trz1[:, 0:32], p_ghn[:], Op.mult)
                n_arg = rsb.tile([128, 32], F32, tag="n_arg")
                nc.vector.tensor_tensor(n_arg[:], vv[:], p_gin[:], Op.add)
                n_g = rsb.tile([128, 32], F32, tag="n_g")
                nc.scalar.activation(n_g[:], n_arg[:], AF.Tanh)
                zm = rsb.tile([128, 32], F32, tag="zm")
                nc.vector.tensor_scalar(zm[:], t_rz[:, 32:64], -0.5, 0.5, Op.mult, Op.add)
                w1 = rsb.tile([128, 32], F32, tag="w1")
                nc.vector.tensor_tensor(
                    w1[:].rearrange("p (c b) -> p c b", c=KD),
                    hist[:].rearrange("p (c tb) -> p c tb", c=KD)
                    [:, :, 8 * t: 8 * t + 8],
                    trz1[:, 32:64].rearrange("p (c b) -> p c b", c=KD), Op.mult)
                # h_new tail uses zm/w1 computed under the ACT tanh
                # h_new = n*(1-z') + h*z' -> written straight into bf16 history
                u_g = rsb.tile([128, 32], F32, tag="u_g")
                nc.vector.tensor_tensor(u_g[:], n_g[:], zm[:], Op.mult)
                nc.vector.tensor_tensor(
                    hist[:].rearrange("p (c tb) -> p c tb", c=KD)
                    [:, :, 8 * (t + 1): 8 * (t + 1) + 8],
                    u_g[:].rearrange("p (c b) -> p c b", c=KD),
                    w1[:].rearrange("p (c b) -> p c b", c=KD), Op.add)

        # ------------------------------ fc phase ------------------------------
        nrem = V - (NCK - 1) * NV  # last tile width (10000 = 19*512 + 272)
        with tc.tile_pool(name="fsb", bufs=4) as fsb, \
             tc.tile_pool(name="fps", bufs=6, space="PSUM") as fps:
            n_mblk = (NT + 99) // 100
            for nck in range(NCK if do_fc else 0):
                nv = NV if nck < NCK - 1 else nrem
                for m in range(n_mblk):
                    mm = min(100, NT - m * 100)
                    p = fps.tile([128, NV], F32, tag="p_fc")
                    for kc in range(KD):
                        nc.tensor.matmul(
                            p[0:mm, 0:nv],
                            hist[:, kc * hcol + 8 + 100 * m: kc * hcol + 8 + 100 * m + mm],
                            wfc_sb[:, kc * V + nck * NV: kc * V + nck * NV + nv],
                            start=(kc == 0), stop=(kc == KD - 1))
                    lg = fsb.tile([128, NV], BF16, tag="lg")
                    nc.vector.tensor_tensor(
                        lg[0:mm, 0:nv], p[0:mm, 0:nv],
                        bfcb[0:mm, nck * NV: nck * NV + nv], Op.add)
                    nc.sync.dma_start(
                        d_logits[m * 100: m * 100 + mm, nck * NV: nck * NV + nv],
                        lg[0:mm, 0:nv])

    nc.finalize()
    return nc


# ------------------------------ host-side prep ------------------------------

def _chunk_lhs(w, k):
    """[K, M] -> [128, (K/128)*M] with col = kc*M + m."""
    K, M = w.shape
    return np.ascontiguousarray(w.reshape(k, 128, M).transpose(1, 0, 2).reshape(128, k * M))


def _chunk_lhs_sq(w, k, mchunks):
    """[K, M] -> [128, k*mchunks*128] with col = (kc*mchunks+mc)*128 + j."""
    K, M = w.shape
    return np.ascontiguousarray(
        w.reshape(k, 128, mchunks, 128).transpose(1, 0, 2, 3).reshape(128, k * mchunks * 128))


def _bf(x):
    return np.ascontiguousarray(x.astype(ml_dtypes.bfloat16))


def host_prep(inputs, n_steps=T):
    i = {k: np.asarray(v) for k, v in inputs.items()}
    sf = i["spatial_feats"].astype(np.float32)          # [64, 49, 2048]
    cap = i["captions"].astype(np.int64)                # [64, 50]
    W_feat, b_feat = i["W_feat"].astype(np.float32), i["b_feat"].astype(np.float32)
    W_ea, b_ea = i["W_ea"].astype(np.float32), i["b_ea"].astype(np.float32)
    W_da, b_da = i["W_da"].astype(np.float32), i["b_da"].astype(np.float32)
    W_fa = i["W_fa"].astype(np.float32)
    emb = i["emb"].astype(np.float32)
    W_ih, W_hh = i["W_ih"].astype(np.float32), i["W_hh"].astype(np.float32)
    b_ih, b_hh = i["b_ih"].astype(np.float32), i["b_hh"].astype(np.float32)
    W_fc, b_fc = i["W_fc"].astype(np.float32), i["b_fc"].astype(np.float32)

    shared = {
        "wfeat": _bf(_chunk_lhs(W_feat, KE)),
        "wea": _bf(_chunk_lhs(W_ea, KD)),
        "wihe": _bf(_chunk_lhs(np.ascontiguousarray(W_ih[:, :EMB].T), KD)),
        "wihc": _bf(_chunk_lhs_sq(np.ascontiguousarray(W_ih[:, EMB:].T), KD, MG)),
        "whh": _bf(_chunk_lhs_sq(np.ascontiguousarray(W_hh.T), KD, MG)),
        "wda": _bf(_chunk_lhs_sq(W_da, KD, MA)),
        "wfa": _bf(W_fa.reshape(MA, 128).T),
        "wfc": _bf(W_fc.reshape(KD, 128, V).transpose(1, 0, 2).reshape(128, KD * V)),
        "bfeat": np.ascontiguousarray(b_feat.reshape(KD, 128).T),
        "bea": np.ascontiguousarray((b_ea + b_da).reshape(MA, 128).T),
        "biasgi": np.ascontiguousarray(
            (b_ih + np.concatenate([b_hh[:2 * DEC], np.zeros(DEC, np.float32)])).reshape(MG, 128).T),
        "bhhnbc": _bf(
            np.repeat(b_hh[2 * DEC:].reshape(4, 128).T[:, :, None], BL, axis=2).reshape(128, 4 * BL)),
        "bfc": _bf(b_fc.reshape(1, V)),
    }
    in_maps = []
    for c in range(NCORES):
        sl = slice(c * BL, (c + 1) * BL)
        sfT = sf[sl].reshape(NL, ENC).T                      # [2048, 392]
        embs = emb[cap[sl][:, :n_steps]]                     # [8, n_steps, 512]
        embT = embs.transpose(1, 0, 2).reshape(BL * n_steps, EMB).T   # [512, NT]
        m = dict(shared)
        m["spatialT"] = _bf(sfT.reshape(KE, 128, NL).transpose(1, 0, 2).reshape(128, KE * NL))
        m["embT"] = _bf(embT.reshape(KD, 128, BL * n_steps).transpose(1, 0, 2).reshape(128, KD * BL * n_steps))
        in_maps.append(m)
    return in_maps


_PROG_CACHE = {}


def _get_prog(n_steps=T):
    if n_steps not in _PROG_CACHE:
        _PROG_CACHE[n_steps] = build_program(n_steps)
    return _PROG_CACHE[n_steps]


def kernel(**inputs):
    from concourse.bass_utils import run_bass_kernel_spmd
    nc = _get_prog(T)
    in_maps = host_prep(inputs, T)
    try:
        res = run_bass_kernel_spmd(nc, in_maps, core_ids=list(range(NCORES)))
    except Exception:
        # transient device errors (e.g. NRT_EXEC_UNIT_UNRECOVERABLE from a
        # previously wedged core) usually clear on retry
        res = run_bass_kernel_spmd(nc, in_maps, core_ids=list(range(NCORES)))
    outs = []
    for c in range(NCORES):
        lg = res.results[c]["logits"]                       # [400, 10000], row = 8t+b
        outs.append(lg.reshape(T, BL, V).transpose(1, 0, 2))  # [8, 50, 10000]
    return np.concatenate(outs, axis=0).astype(np.float32)    # [64, 50, 10000]



# revision 59
# speedup vs baseline: 1.2594x; 1.2594x over previous
"""Trainium2 Bass kernel for nn_CaptionModel (GRU + Bahdanau attention caption decoder).

Sharding: pure data-parallel over batch. B=64 -> 8 cores x 8 rows each; no
collectives (50 sequential steps cannot afford the ~5us/call collective floor).

Per-core plan (feature-major: features on partitions, local batch b=8 on free):
  setup (DMA-order tuned; enc matmuls kc-pipelined behind the chunked
  spatialT/wfeat DMAs; bias-adds split DVE/ACT via Identity-with-bias):
          enc = W_feat.T @ spatialT (+b_feat)        [512, 392]
          att1 = W_ea.T @ enc (+b_ea)                [256, 392] bf16
          enc_bd: block-diag [l, d] pair tiles for the context matmuls
                  (pad rows hold garbage - context matmuls partition-range)
          gi_emb cols [0:104) here; the remaining mc-rows ride the first 12
          steps' idle windows as deferred units
  50 steps, all at a uniform 6.62us serial-chain period:
          gh   = W_hh.T @ h  /  att2 = W_da.T @ h  (weight-stationary bf16)
          tanh(att1 + att2) -> scores = tanhT @ W_fa
          ONE exp over psum rows 0:113 -> masked-ones denominator matmuls
          per-b context matmuls (partition-ranged lhsT) -> x_ctx = ctx/denom
          gi_ctx accumulated onto the gh psums; n-gate's r'*h_n folded in
          via a 0.5*I matmul; gates use fused scalar_tensor_tensor algebra
          (sigmoid(x) = 0.5 tanh(x/2)+0.5, no standalone affines on-chain)
  fc:     logits = H_hist.T @ W_fc vocab-sweep units for 13-step row blocks,
          emitted after the loop with 1-element psum "readiness gates" so the
          tile list-scheduler spreads them across step idle windows (PE would
          otherwise sit ~75% idle); only the final 11-step block runs after
          step 50. b_fc is added host-side.

kernel() accepts FULL inputs, does host-side layout prep/sharding (incl. the
embedding-table gather), runs the same NEFF SPMD on cores 0-7, unshards.
"""

import contextlib

import ml_dtypes
import numpy as np

import concourse.bass as bass
import concourse.mybir as mybir
from concourse import bacc
from concourse.alu_op_type import AluOpType as Op
from concourse.masks import make_identity
from concourse.tile import TileContext

AF = mybir.ActivationFunctionType
F32 = mybir.dt.float32
BF16 = mybir.dt.bfloat16
F32R = mybir.dt.float32r

B, L, ENC, DEC, EMB, ATT, V, T = 64, 49, 2048, 512, 512, 256, 10000, 50
NCORES = 8
BL = B // NCORES          # 8 local batch rows
NL = BL * L               # 392
KE = ENC // 128           # 16 K-chunks for enc matmul
KD = DEC // 128           # 4 K-chunks over DEC
MG = (3 * DEC) // 128     # 12 M-chunks over gates
MA = ATT // 128           # 2 M-chunks over ATT
NPAIR = BL // 2           # 4 block-diag pairs
HCOL = 8 * (T + 1)        # 408 cols per chunk in H history
NV = 512                  # fc vocab tile width
NCK = (V + NV - 1) // NV  # 20 fc vocab tiles (last = 272 wide)


def build_program(n_steps=T, do_rec=True, do_fc=True):
    nc = bacc.Bacc()
    NT = BL * n_steps      # t*8+b columns
    hcol = 8 * (n_steps + 1)

    # ---------------- DRAM I/O (per-core, host-prepped layouts) ----------------
    d_spatialT = nc.dram_tensor("spatialT", [128, KE * NL], BF16, kind="ExternalInput")
    d_embT = nc.dram_tensor("embT", [128, KD * NT], BF16, kind="ExternalInput")
    d_wfeat = nc.dram_tensor("wfeat", [128, KE * DEC], BF16, kind="ExternalInput")
    d_wea = nc.dram_tensor("wea", [128, KD * ATT], BF16, kind="ExternalInput")
    d_wihe = nc.dram_tensor("wihe", [128, KD * 3 * DEC], BF16, kind="ExternalInput")
    d_wihc = nc.dram_tensor("wihc", [128, KD * MG * 128], BF16, kind="ExternalInput")
    d_whh = nc.dram_tensor("whh", [128, KD * MG * 128], BF16, kind="ExternalInput")
    d_wda = nc.dram_tensor("wda", [128, KD * MA * 128], BF16, kind="ExternalInput")
    d_wfa = nc.dram_tensor("wfa", [128, MA], BF16, kind="ExternalInput")
    d_wfc = nc.dram_tensor("wfc", [128, KD * V], BF16, kind="ExternalInput")
    d_bfeat = nc.dram_tensor("bfeat", [128, KD], F32, kind="ExternalInput")
    d_bea = nc.dram_tensor("bea", [128, MA], F32, kind="ExternalInput")
    d_biasgi = nc.dram_tensor("biasgi", [128, MG], F32, kind="ExternalInput")
    d_bhhnbc = nc.dram_tensor("bhhnbc", [128, 4 * BL], BF16, kind="ExternalInput")
    d_logits = nc.dram_tensor("logits", [NT, V], BF16, kind="ExternalOutput")

    with TileContext(nc) as tc, contextlib.ExitStack() as ctx:
        const = ctx.enter_context(tc.tile_pool(name="const", bufs=1))
        state = ctx.enter_context(tc.tile_pool(name="state", bufs=1))

        # persistent weights / constants in SBUF
        wihc = const.tile([128, KD * MG * 128], BF16, tag="wihc")
        whh = const.tile([128, KD * MG * 128], BF16, tag="whh")
        wda = const.tile([128, KD * MA * 128], BF16, tag="wda")
        wfa = const.tile([128, MA], BF16, tag="wfa")
        bhhnbc = const.tile([128, 4 * BL], BF16, tag="bhhnbc")
        ident_f = const.tile([128, 128], BF16, tag="ident_f")
        make_identity(nc, ident_f[:])
        # 0.5*I: accumulates 0.5*vv into the n-gate psum without a DVE op
        ident_h = const.tile([128, 128], BF16, tag="ident_h")
        nc.vector.tensor_scalar(ident_h[:], ident_f[:], 0.5, None, Op.mult)
        # masked all-ones matrices for the softmax denominator: rows 0:49
        # (even-b block) and rows 64:113 (odd-b block); pad rows stay zero so
        # exp(0)=1 pad values in expe never reach the denominator.
        mask_up = const.tile([128, 128], BF16, tag="mask_up")
        mask_lo = const.tile([128, 128], BF16, tag="mask_lo")
        wfc_sb = const.tile([128, KD * V], BF16, tag="wfc_sb")
        nc.gpsimd.memset(mask_up[:], 0.0)
        nc.gpsimd.memset(mask_lo[:], 0.0)
        nc.gpsimd.memset(mask_up[0:L, :], 1.0)
        nc.gpsimd.memset(mask_lo[64:64 + L, :], 1.0)

        # persistent activations / state
        att1 = state.tile([128, MA * NL], BF16, tag="att1")
        # block-diag context tiles, c-major: col (c*NPAIR + j)*128; pad rows
        # 49:64 / 113:128 hold garbage (never read by the partition-ranged
        # context matmuls)
        enc_bd = state.tile([128, KD * NPAIR * 128], BF16, tag="enc_bd")
        gi_emb = state.tile([128, MG * NT], BF16, tag="gi_emb")
        hist = state.tile([128, KD * hcol], BF16, tag="hist")
        expe = state.tile([128, NPAIR], BF16, tag="expe")
        tanh_sb = state.tile([128, MA * BL * 64], BF16, tag="tanh_sb")
        # embT/wihe persist into the recurrence: 3/4 of the gi_emb matmuls
        # are deferred into the first steps' idle PE windows.
        embT = state.tile([128, KD * NT], BF16, tag="embT")
        wihe = state.tile([128, KD * 3 * DEC], BF16, tag="wihe")
        biasgi = const.tile([128, MG], F32, tag="biasgi")
        nc.gpsimd.memset(tanh_sb[:], 0.0)
        nc.gpsimd.memset(hist[:], 0.0)
        nc.gpsimd.memset(expe[:], 0.0)

        # gi_emb column split: [0:GIE0) in setup, rest deferred into steps
        GIE0 = min(8 * 13, NT)

        # ------------------------------ setup phase ------------------------------
        with tc.tile_pool(name="ssb", bufs=1) as ssb, \
             tc.tile_pool(name="sps", bufs=1, space="PSUM") as sps:
            spatialT = ssb.tile([128, KE * NL], BF16, tag="spatialT")
            wfeat = ssb.tile([128, KE * DEC], BF16, tag="wfeat")
            wea = ssb.tile([128, KD * ATT], BF16, tag="wea")
            bfeat = ssb.tile([128, KD], F32, tag="bfeat")
            bea = ssb.tile([128, MA], F32, tag="bea")
            # DMA order is the setup critical path (one shared DMA device):
            # spatialT/wfeat chunks first, interleaved, so the kc-pipelined
            # enc matmuls start after the first chunk pair lands; everything
            # the recurrence needs next; wfc (needed from step ~13) last.
            for g in range(0, KE, 4):
                nc.sync.dma_start(spatialT[:, g * NL: (g + 4) * NL],
                                  d_spatialT[:, g * NL: (g + 4) * NL])
                nc.sync.dma_start(wfeat[:, g * DEC: (g + 4) * DEC],
                                  d_wfeat[:, g * DEC: (g + 4) * DEC])
            for dst, src in [(bfeat, d_bfeat), (bea, d_bea), (embT, d_embT),
                             (wihe, d_wihe), (wea, d_wea), (biasgi, d_biasgi),
                             (whh, d_whh), (wda, d_wda), (wfa, d_wfa),
                             (bhhnbc, d_bhhnbc), (wihc, d_wihc)]:
                nc.sync.dma_start(dst[:], src[:])

            enc_fm = ssb.tile([128, KD * NL], BF16, tag="enc_fm")
            # enc = W_feat.T @ spatialT (+ b_feat), kc-outer so each chunk's
            # matmuls run as soon as its DMA pair lands (4 psum banks, one
            # per output chunk)
            p_enc = [sps.tile([128, NL], F32, tag=f"p_enc{mc}", name=f"p_enc{mc}")
                     for mc in range(KD)]

            for kc in range(KE):
                for mc in range(KD):
                    nc.tensor.matmul(
                        p_enc[mc][:],
                        wfeat[:, kc * DEC + mc * 128: kc * DEC + mc * 128 + 128],
                        spatialT[:, kc * NL: (kc + 1) * NL],
                        start=(kc == 0), stop=(kc == KE - 1))
            for mc in range(KD):
                if mc % 2 == 0:
                    nc.vector.tensor_scalar(
                        enc_fm[:, mc * NL: (mc + 1) * NL], p_enc[mc][:],
                        bfeat[:, mc: mc + 1], None, Op.add)
                else:
                    nc.scalar.activation(
                        enc_fm[:, mc * NL: (mc + 1) * NL], p_enc[mc][:],
                        AF.Identity, bias=bfeat[:, mc: mc + 1])

            # att1 = W_ea.T @ enc (+ b_ea)  -> bf16 [att-chunk, b*49+l]
            for mc in range(MA):
                p = sps.tile([128, NL], F32, tag="p_att1")
                for kc in range(KD):
                    nc.tensor.matmul(
                        p[:],
                        wea[:, kc * ATT + mc * 128: kc * ATT + mc * 128 + 128],
                        enc_fm[:, kc * NL: (kc + 1) * NL],
                        start=(kc == 0), stop=(kc == KD - 1))
                if mc == 0:
                    nc.vector.tensor_scalar(
                        att1[:, mc * NL: (mc + 1) * NL], p[:],
                        bea[:, mc: mc + 1], None, Op.add)
                else:
                    nc.scalar.activation(
                        att1[:, mc * NL: (mc + 1) * NL], p[:],
                        AF.Identity, bias=bea[:, mc: mc + 1])

            # gi_emb = W_ihe.T @ embT (+ b_ih + [b_hh folded for r,z]);
            # only the first GIE0 columns here, the rest mid-recurrence.
            # Emitted before the enc_bd transposes: it gates step 0, they
            # only gate the first context matmuls (~4us into step 0).
            for mc in range(MG):
                p = sps.tile([128, GIE0], F32, tag="p_gie", bufs=2)
                for kc in range(KD):
                    nc.tensor.matmul(
                        p[:],
                        wihe[:, kc * 3 * DEC + mc * 128: kc * 3 * DEC + mc * 128 + 128],
                        embT[:, kc * NT: kc * NT + GIE0],
                        start=(kc == 0), stop=(kc == KD - 1))
                if mc % 2 == 0:
                    nc.vector.tensor_scalar(
                        gi_emb[:, mc * NT: mc * NT + GIE0], p[:],
                        biasgi[:, mc: mc + 1], None, Op.add)
                else:
                    nc.scalar.activation(
                        gi_emb[:, mc * NT: mc * NT + GIE0], p[:],
                        AF.Identity, bias=biasgi[:, mc: mc + 1])

            # enc_bd: rows 0:49 = enc[b=2j] (l, d); rows 64:113 = enc[b=2j+1]
            # (psum/ACT partition bases must be 0/32/64 -> 64-offset pairing).
            # Two adjacent pairs share one psum->sbuf copy (pad rows carry
            # psum garbage, never read); copies alternate DVE/ACT.
            pt = sps.tile([128, 512], F32, tag="p_tr")
            grp = 0
            for c in range(KD):
                for j2 in range(0, NPAIR, 2):
                    sb0 = 256 * (grp % 2)
                    for dj in range(2):
                        for s in range(2):
                            b = 2 * (j2 + dj) + s
                            base = 64 * s
                            nc.tensor.matmul(
                                pt[base: base + L, sb0 + dj * 128: sb0 + dj * 128 + 128],
                                enc_fm[:, c * NL + b * L: c * NL + b * L + L],
                                ident_f[:], start=True, stop=True)
                    dst = enc_bd[0: 64 + L,
                                 (c * NPAIR + j2) * 128: (c * NPAIR + j2) * 128 + 256]
                    if grp % 2 == 0:
                        nc.vector.tensor_copy(dst, pt[0: 64 + L, sb0: sb0 + 256])
                    else:
                        nc.scalar.activation(dst, pt[0: 64 + L, sb0: sb0 + 256], AF.Copy)
                    grp += 1

        # ------------------------------ recurrence ------------------------------
        nc.sync.dma_start(wfc_sb[:], d_wfc[:])

        # fc interleave: vocab-sweep units for completed 13-step row blocks
        # are injected into the PE's idle windows inside the recurrence (the
        # tensor engine sits ~75% idle waiting on the serial attention/gate
        # chain). The final block has to run after the loop.
        fc_blocks = [(s, min(s + 13, n_steps)) for s in range(0, n_steps, 13)] if do_fc else []
        inter_blocks = fc_blocks[:-1] if do_rec and len(fc_blocks) > 1 else []
        tail_blocks = fc_blocks[len(inter_blocks):]
        fc_units = [(s0, s1, nck) for (s0, s1) in inter_blocks for nck in range(NCK)]
        nrem = V - (NCK - 1) * NV  # last tile width (10000 = 19*512 + 272)

        with tc.tile_pool(name="rsb", bufs=3) as rsb, \
             tc.tile_pool(name="rps", bufs=1, space="PSUM") as rps, \
             tc.tile_pool(name="fsb", bufs=4) as fsb, \
             tc.tile_pool(name="fps", bufs=4, space="PSUM") as fps:

            # all small per-step psum tiles share one bank (disjoint column
            # ranges; PSUM accumulation is address-granular), and p_ghrz
            # double-buffers inside a second bank, leaving 4 banks for the
            # interleaved fc units.
            p_misc = rps.tile([128, 128], F32, tag="p_misc")
            p_ghrz2 = rps.tile([128, 128], F32, tag="p_ghrz2")

            def fc_mms(s0, s1, nck, gate_step=None):
                rows = 8 * (s1 - s0)
                col0 = 8 * (s0 + 1)
                nv = NV if nck < NCK - 1 else nrem
                p = fps.tile([128, NV], F32, tag="p_fc")
                if gate_step is not None:
                    # readiness throttle for the list scheduler: a 1-element
                    # write into the matmul's psum range that depends on a
                    # later h column staggers when this unit can start
                    # (overwritten by the start=True matmul below)
                    gc = 8 * (gate_step + 1)
                    nc.vector.tensor_copy(p[0:1, 0:1], hist[0:1, gc: gc + 1])
                for kc in range(KD):
                    nc.tensor.matmul(
                        p[0:rows, 0:nv],
                        hist[:, kc * hcol + col0: kc * hcol + col0 + rows],
                        wfc_sb[:, kc * V + nck * NV: kc * V + nck * NV + nv],
                        start=(kc == 0), stop=(kc == KD - 1))
                return p

            def fc_flush(p, s0, s1, nck):
                # b_fc is added host-side; this is just the psum->bf16 move
                rows = 8 * (s1 - s0)
                nv = NV if nck < NCK - 1 else nrem
                lg = fsb.tile([128, NV], BF16, tag="lg")
                nc.vector.tensor_copy(lg[0:rows, 0:nv], p[0:rows, 0:nv])
                nc.sync.dma_start(
                    d_logits[8 * s0: 8 * s0 + rows, nck * NV: nck * NV + nv],
                    lg[0:rows, 0:nv])

            def gie_unit(mc, t):
                # one deferred gi_emb mc-row, emitted inside step t's body;
                # the readiness gate (a 1-element psum write depending on
                # h_t) keeps the scheduler from front-loading it before t
                p = fps.tile([128, NV], F32, tag="p_fc")
                nc.vector.tensor_copy(p[0:1, 0:1], hist[0:1, 8 * t: 8 * t + 1])
                for kc in range(KD):
                    nc.tensor.matmul(
                        p[:, 0: NT - GIE0],
                        wihe[:, kc * 3 * DEC + mc * 128: kc * 3 * DEC + mc * 128 + 128],
                        embT[:, kc * NT + GIE0: (kc + 1) * NT],
                        start=(kc == 0), stop=(kc == KD - 1))
                # bias-add on ACT (Identity is in the loaded exp/tanh table
                # set) -- keeps the busier DVE out of these deferred units
                nc.scalar.activation(
                    gi_emb[:, mc * NT + GIE0: (mc + 1) * NT], p[:, 0: NT - GIE0],
                    AF.Identity, bias=biasgi[:, mc: mc + 1])

            for t in range(n_steps if do_rec else 0):
                hprev = [hist[:, kc * hcol + 8 * t: kc * hcol + 8 * t + 8] for kc in range(KD)]

                # h05 = 0.5*h, consumed by the z'*h fused op late in the step;
                # issued first so it runs while the PE works on att2/gh.
                h05 = rsb.tile([128, 32], F32, tag="h05")
                nc.vector.tensor_scalar(
                    h05[:].rearrange("p (c b) -> p c b", c=KD),
                    hist[:].rearrange("p (c tb) -> p c tb", c=KD)
                    [:, :, 8 * t: 8 * t + 8],
                    0.5, None, Op.mult)

                # gh (r,z and n) and att2, weight-stationary bf16. The gi_emb
                # slice and b_hh_n enter psum via identity matmuls (frees the
                # DVE pre-add chain; ACT reads gates straight from psum).
                gie = gi_emb[:].rearrange("p (mc tb) -> p mc tb", mc=MG)
                p_ghrz = p_ghrz2[:, 64 * (t % 2): 64 * (t % 2) + 64]
                p_att2 = p_misc[:, 0:16]
                p_sc = p_misc[:, 16:20]
                p_small = p_misc[:, 20:28]
                p_ctx = p_misc[:, 28:60]
                p_ghn = p_misc[:, 60:92]
                p_gin = p_misc[:, 92:124]
                for mc in range(MA):
                    for kc in range(KD):
                        nc.tensor.matmul(
                            p_att2[:, mc * 8: mc * 8 + 8],
                            wda[:, (kc * MA + mc) * 128: (kc * MA + mc) * 128 + 128],
                            hprev[kc], start=(kc == 0), stop=(kc == KD - 1))
                nc.tensor.matmul(
                    p_ghrz, ident_f[:],
                    gie[:, 0:8, 8 * t: 8 * t + 8],
                    start=True, stop=False)
                nc.tensor.matmul(
                    p_ghn, ident_f[:],
                    bhhnbc[:],
                    start=True, stop=False)
                for mc in range(8):
                    for kc in range(KD):
                        nc.tensor.matmul(
                            p_ghrz[:, mc * 8: mc * 8 + 8],
                            whh[:, (kc * MG + mc) * 128: (kc * MG + mc) * 128 + 128],
                            hprev[kc], start=False, stop=False)
                for mc in range(8, MG):
                    for kc in range(KD):
                        nc.tensor.matmul(
                            p_ghn[:, (mc - 8) * 8: (mc - 8) * 8 + 8],
                            whh[:, (kc * MG + mc) * 128: (kc * MG + mc) * 128 + 128],
                            hprev[kc], start=False,
                            stop=(kc == KD - 1 and mc == MG - 1))

                # tanh(att1 + att2 broadcast over l); b_da+b_ea pre-folded into
                # att1. Split by att-chunk so the second TT overlaps the first
                # tanh (shorter critical path into the score matmuls).
                targ = rsb.tile([128, MA * NL], BF16, tag="targ")
                for c in range(MA):
                    nc.vector.tensor_tensor(
                        targ[:, c * NL: (c + 1) * NL].rearrange(
                            "p (b l) -> p b l", b=BL, l=L),
                        att1[:, c * NL: (c + 1) * NL].rearrange(
                            "p (b l) -> p b l", b=BL, l=L),
                        p_att2[:, c * BL: (c + 1) * BL].unsqueeze(2)
                        .broadcast_to([128, BL, L]),
                        Op.add)
                    nc.scalar.activation(
                        tanh_sb[:, c * 512: (c + 1) * 512].rearrange(
                            "p (b l) -> p b l", b=BL, l=64)[:, :, 0:L],
                        targ[:, c * NL: (c + 1) * NL].rearrange(
                            "p (b l) -> p b l", b=BL, l=L),
                        AF.Tanh)

                # scores -> psum [128 rows, pair cols]: one MM per (pair, kc);
                # lhsT is the 64-stride padded pair block (odd b -> rows 64+)
                for j in range(NPAIR):
                    for kc in range(MA):
                        nc.tensor.matmul(
                            p_sc[:, j: j + 1],
                            tanh_sb[:, kc * 512 + j * 128: kc * 512 + j * 128 + 128],
                            wfa[:, kc: kc + 1],
                            start=(kc == 0), stop=(kc == MA - 1))

                # ONE exp over rows 0:113 (pad rows 49:64 become exp(0)=1 but
                # are never read: the masked denominator matmuls and the
                # per-b context matmuls skip them via zero mask rows /
                # partition-ranged lhsT).
                nc.scalar.activation(expe[0:64 + L, 0:NPAIR], p_sc[0:64 + L, 0:NPAIR], AF.Exp)

                # denominator broadcast to all partitions: two masked-ones
                # matmuls; cols 0:4 = even b, cols 4:8 = odd b.
                nc.tensor.matmul(p_small[:, 0:NPAIR], mask_up[:], expe[:], start=True, stop=True)
                nc.tensor.matmul(p_small[:, NPAIR:BL], mask_lo[:], expe[:], start=True, stop=True)

                # context: one matmul per (b, chunk) with partition-ranged lhsT
                for b in range(BL):
                    base = 64 * (b % 2)
                    for c in range(KD):
                        nc.tensor.matmul(
                            p_ctx[:, c * BL + b: c * BL + b + 1],
                            enc_bd[base: base + L,
                                   (c * NPAIR + b // 2) * 128: (c * NPAIR + b // 2) * 128 + 128],
                            expe[base: base + L, b // 2: b // 2 + 1],
                            start=True, stop=True)
                rb_sb = rsb.tile([128, BL], F32, tag="rb_sb")
                nc.vector.reciprocal(rb_sb[:], p_small)
                # x_ctx = ctx * (1/denom); rb cols are (parity, pair)-ordered
                x_ctx = rsb.tile([128, KD * BL], BF16, tag="x_ctx")
                nc.vector.tensor_tensor(
                    x_ctx[:].rearrange("p (c j s) -> p c s j", c=KD, j=NPAIR, s=2),
                    p_ctx.rearrange("p (c j s) -> p c s j", c=KD, j=NPAIR, s=2),
                    rb_sb[:].rearrange("p (s j) -> p s j", s=2, j=NPAIR)
                    .unsqueeze(1).broadcast_to([128, KD, 2, NPAIR]),
                    Op.mult)

                # gi_ctx: r,z accumulate onto p_ghrz; n into p_gin (pre-loaded
                # with the gi_emb n-slice via identity matmul)
                nc.tensor.matmul(
                    p_gin, ident_f[:],
                    gie[:, 8:MG, 8 * t: 8 * t + 8],
                    start=True, stop=False)
                for mc in range(8):
                    for kc in range(KD):
                        nc.tensor.matmul(
                            p_ghrz[:, mc * 8: mc * 8 + 8],
                            wihc[:, (kc * MG + mc) * 128: (kc * MG + mc) * 128 + 128],
                            x_ctx[:, kc * 8: kc * 8 + 8], start=False,
                            stop=(kc == KD - 1 and mc == 7))
                for mc in range(8, MG):
                    for kc in range(KD):
                        nc.tensor.matmul(
                            p_gin[:, (mc - 8) * 8: (mc - 8) * 8 + 8],
                            wihc[:, (kc * MG + mc) * 128: (kc * MG + mc) * 128 + 128],
                            x_ctx[:, kc * 8: kc * 8 + 8], start=False, stop=False)

                # gates: t_rz = tanh(0.5 * rz_full) straight from psum.
                # sigmoid(x) = 0.5*tanh(x/2) + 0.5 is folded algebraically
                # into fused scalar_tensor_tensor ops so no standalone affine
                # sits on the critical chain:
                #   n_arg = r'*h_n + g_in = 0.5*((t_r + 1)*h_n) + g_in
                #   h'    = (1-z')*n + z'*h = -0.5*((t_z - 1)*n) + z'*h
                t_rz = rsb.tile([128, 64], F32, tag="t_rz")
                nc.scalar.activation(t_rz[:], p_ghrz, AF.Tanh, scale=0.5)
                # vv = (t_r + 1)*h_n = 2*r'*h_n, accumulated as 0.5*vv into
                # the open p_gin group via the half-identity matmul (closing
                # it); the n-gate tanh then reads the psum directly.
                vv = rsb.tile([128, 32], BF16, tag="vv")
                nc.vector.scalar_tensor_tensor(
                    vv[:], t_rz[:, 0:32], 1.0, p_ghn, Op.add, Op.mult)
                nc.tensor.matmul(p_gin, ident_h[:], vv[:], start=False, stop=True)
                n_g = rsb.tile([128, 32], F32, tag="n_g")
                nc.scalar.activation(n_g[:], p_gin, AF.Tanh)
                # w1 = z'*h = 0.5*(t_z+1)*h, via h05 = 0.5*h computed at step
                # start; runs on DVE under the ACT tanh of the n-gate.
                w1 = rsb.tile([128, 32], F32, tag="w1")
                nc.vector.scalar_tensor_tensor(
                    w1[:].rearrange("p (c b) -> p c b", c=KD),
                    t_rz[:, 32:64].rearrange("p (c b) -> p c b", c=KD),
                    1.0, h05[:].rearrange("p (c b) -> p c b", c=KD),
                    Op.add, Op.mult)
                # h_new = -0.5*((t_z-1)*n) + w1 -> bf16 history
                u_g = rsb.tile([128, 32], F32, tag="u_g")
                nc.vector.scalar_tensor_tensor(
                    u_g[:], t_rz[:, 32:64], 1.0, n_g[:], Op.subtract, Op.mult)
                nc.vector.scalar_tensor_tensor(
                    hist[:].rearrange("p (c tb) -> p c tb", c=KD)
                    [:, :, 8 * (t + 1): 8 * (t + 1) + 8],
                    u_g[:].rearrange("p (c b) -> p c b", c=KD),
                    -0.5,
                    w1[:].rearrange("p (c b) -> p c b", c=KD), Op.mult, Op.add)

                # deferred gi_emb work rides at the END of the step body so
                # its scheduler priority sits behind this step's chain ops
                if GIE0 < NT and 1 <= t <= MG:
                    gie_unit(t - 1, t)

            # ----------------- fc units (scheduler-placed) -----------------
            # interleaved units, readiness staggered at 2 vocab-tiles per
            # step so the scheduler spreads them across idle windows
            for k, (s0, s1, nck) in enumerate(fc_units):
                gate = min(s1 + (k % NCK) // 2, n_steps - 1)
                fc_flush(fc_mms(s0, s1, nck, gate_step=gate), s0, s1, nck)
            for (s0, s1) in tail_blocks:
                for nck in range(NCK):
                    fc_flush(fc_mms(s0, s1, nck), s0, s1, nck)

    nc.finalize()
    return nc


# ------------------------------ host-side prep ------------------------------

def _chunk_lhs(w, k):
    """[K, M] -> [128, (K/128)*M] with col = kc*M + m."""
    K, M = w.shape
    return np.ascontiguousarray(w.reshape(k, 128, M).transpose(1, 0, 2).reshape(128, k * M))


def _chunk_lhs_sq(w, k, mchunks):
    """[K, M] -> [128, k*mchunks*128] with col = (kc*mchunks+mc)*128 + j."""
    K, M = w.shape
    return np.ascontiguousarray(
        w.reshape(k, 128, mchunks, 128).transpose(1, 0, 2, 3).reshape(128, k * mchunks * 128))


def _bf(x):
    return np.ascontiguousarray(x.astype(ml_dtypes.bfloat16))


def host_prep(inputs, n_steps=T):
    i = {k: np.asarray(v) for k, v in inputs.items()}
    sf = i["spatial_feats"].astype(np.float32)          # [64, 49, 2048]
    cap = i["captions"].astype(np.int64)                # [64, 50]
    W_feat, b_feat = i["W_feat"].astype(np.float32), i["b_feat"].astype(np.float32)
    W_ea, b_ea = i["W_ea"].astype(np.float32), i["b_ea"].astype(np.float32)
    W_da, b_da = i["W_da"].astype(np.float32), i["b_da"].astype(np.float32)
    W_fa = i["W_fa"].astype(np.float32)
    emb = i["emb"].astype(np.float32)
    W_ih, W_hh = i["W_ih"].astype(np.float32), i["W_hh"].astype(np.float32)
    b_ih, b_hh = i["b_ih"].astype(np.float32), i["b_hh"].astype(np.float32)
    W_fc, b_fc = i["W_fc"].astype(np.float32), i["b_fc"].astype(np.float32)

    shared = {
        "wfeat": _bf(_chunk_lhs(W_feat, KE)),
        "wea": _bf(_chunk_lhs(W_ea, KD)),
        "wihe": _bf(_chunk_lhs(np.ascontiguousarray(W_ih[:, :EMB].T), KD)),
        "wihc": _bf(_chunk_lhs_sq(np.ascontiguousarray(W_ih[:, EMB:].T), KD, MG)),
        "whh": _bf(_chunk_lhs_sq(np.ascontiguousarray(W_hh.T), KD, MG)),
        "wda": _bf(_chunk_lhs_sq(W_da, KD, MA)),
        "wfa": _bf(W_fa.reshape(MA, 128).T),
        "wfc": _bf(W_fc.reshape(KD, 128, V).transpose(1, 0, 2).reshape(128, KD * V)),
        "bfeat": np.ascontiguousarray(b_feat.reshape(KD, 128).T),
        "bea": np.ascontiguousarray((b_ea + b_da).reshape(MA, 128).T),
        "biasgi": np.ascontiguousarray(
            (b_ih + np.concatenate([b_hh[:2 * DEC], np.zeros(DEC, np.float32)])).reshape(MG, 128).T),
        "bhhnbc": _bf(
            np.repeat(b_hh[2 * DEC:].reshape(4, 128).T[:, :, None], BL, axis=2).reshape(128, 4 * BL)),
    }
    in_maps = []
    for c in range(NCORES):
        sl = slice(c * BL, (c + 1) * BL)
        sfT = sf[sl].reshape(NL, ENC).T                      # [2048, 392]
        embs = emb[cap[sl][:, :n_steps]]                     # [8, n_steps, 512]
        embT = embs.transpose(1, 0, 2).reshape(BL * n_steps, EMB).T   # [512, NT]
        m = dict(shared)
        m["spatialT"] = _bf(sfT.reshape(KE, 128, NL).transpose(1, 0, 2).reshape(128, KE * NL))
        m["embT"] = _bf(embT.reshape(KD, 128, BL * n_steps).transpose(1, 0, 2).reshape(128, KD * BL * n_steps))
        in_maps.append(m)
    return in_maps


_PROG_CACHE = {}


def _get_prog(n_steps=T):
    if n_steps not in _PROG_CACHE:
        _PROG_CACHE[n_steps] = build_program(n_steps)
    return _PROG_CACHE[n_steps]


def kernel(**inputs):
    from concourse.bass_utils import run_bass_kernel_spmd
    nc = _get_prog(T)
    in_maps = host_prep(inputs, T)
    try:
        res = run_bass_kernel_spmd(nc, in_maps, core_ids=list(range(NCORES)))
    except Exception:
        # transient device errors (e.g. NRT_EXEC_UNIT_UNRECOVERABLE from a
        # previously wedged core) usually clear on retry
        res = run_bass_kernel_spmd(nc, in_maps, core_ids=list(range(NCORES)))
    outs = []
    for c in range(NCORES):
        lg = res.results[c]["logits"]                       # [400, 10000], row = 8t+b
        outs.append(lg.reshape(T, BL, V).transpose(1, 0, 2))  # [8, 50, 10000]
    full = np.concatenate(outs, axis=0).astype(np.float32)    # [64, 50, 10000]
    return full + np.asarray(inputs["b_fc"]).astype(np.float32)[None, None, :]

